# revision 13
# baseline (speedup 1.0000x reference)
"""Bass/Trainium2 kernel for nn_BboxIoULoss (topk_masking).

Computes, for S=64 samples / M=1024 targets / P=8256 triu proposals:
    loss = sum((1 - diou) * mask) / sum(mask)
where mask = topk-scatter(3) OR (iou1ds > 0.5), iou1ds = triu-gather of
iou2ds, and diou is the 1-D DIoU between each target and the per-sample
proposal moments.

Strategy (8 NeuronCores, data-parallel over M):
  - core k handles targets m in [128k, 128(k+1)) -> 128 partitions = targets.
  - iou2ds shard is loaded triu-row-wise directly into "p-order" columns,
    so column p of the on-chip tile pairs with out_moments[s, p].
  - out_moments shard (8 samples) is loaded compactly as [128, 1032] f32,
    converted to bf16 s1/e1 [128, 516] on-chip, bounced through a DRAM
    scratch, and read back replicated 16x across partitions so that
    partition m holds sample floor(m/16).
  - per-chunk DVE/ACT pipeline (bf16) with per-partition tgt scalars:
        w     = min(e1,e2) - max(s1,s2)
        enc   = relu(e1-e2) + relu(s2-s1) + (e2-s2)    # enclose length
        inter = relu(w)
        renc  = 1/enc  (RECIPROCAL_APPROX_FAST, ~51 ULP)
        mask  = iou1 > 0.5   (exact fp32 compare)
        rm    = mask * renc
        A  += sum(mask); B1 += sum(inter*rm); B2q += sum(((s1+e1-c2)*rm)^2)
    Using mask^2 = mask:  sum(mask*diou) = B1 - B2q/4, so
        answer = (A - B1 + B2q/4) / A.
  - the top-3 scatter is subsumed by the threshold whenever every row has
    >= 3 entries above 0.5 (then the top-3 values are all > 0.5). The
    device returns per-row counts; if any row has < 3, or num_targets is
    not uniform, a numpy fallback reproduces the reference exactly.
"""

import os
import numpy as np

import concourse.bass as bass
import concourse.tile as tile
import concourse.mybir as mybir
from concourse import bacc, bass_utils

F32 = mybir.dt.float32
BF16 = mybir.dt.bfloat16
U16 = mybir.dt.uint16
AF = mybir.ActivationFunctionType
OP = mybir.AluOpType

S = 64
T = 16
N = 128
M = S * T                  # 1024
P = N * (N + 1) // 2       # 8256
TOPK = 3
IOU_THRESHOLD = 0.5
NCORES = 8
ML = M // NCORES           # 128 targets / core  (= partitions)
W = S // NCORES            # 8 samples / core
CH = 1032                  # p-chunk (P = 8 * 1032)
NCH = P // CH
PK = P // T                # 516
RECIP_C0 = -0.23549792     # Chebyshev seed scale for ~x reciprocal trick
RECIP_C1 = 2.0017324
RECIP_BIAS = 1.00138       # host-side correction for 1-NR underestimate


def _triu_rows():
    """(i, p_offset, row_len) for each triangle row in p-order."""
    rows = []
    off = 0
    for i in range(N):
        rows.append((i, off, N - i))
        off += N - i
    return rows


def _build_program():
    nc = bacc.Bacc(
        "TRN2", target_bir_lowering=False, debug=False, enable_asserts=False
    )
    # Host passes pre-reshaped shards:
    #   iou: [ML, N*N] f32   (iou2ds[m] flattened row-major)
    #   omc: [128, 1032] f32 (out_moments[8, 8256, 2] flat; row s*16+k holds
    #                         interleaved (s,e) pairs for p in [516k, 516(k+1)))
    #   tgt: [ML, 2]    f32
    iou_d = nc.dram_tensor("iou", [ML, N * N], F32, kind="ExternalInput")
    omc_d = nc.dram_tensor("omc", [128, 2 * PK], F32, kind="ExternalInput")
    tgt_d = nc.dram_tensor("tgt", [ML, 2], F32, kind="ExternalInput")
    # acc: cols [0:NCH]=A, [NCH:2NCH]=B1, [2NCH:3NCH]=B2q  (per-chunk sums)
    acc_d = nc.dram_tensor("acc", [ML, 3 * NCH], F32, kind="ExternalOutput")

    rows = _triu_rows()

    linearize = bool(int(os.environ.get("BBK_LINEARIZE", "0")))
    with tile.TileContext(nc, linearize=linearize) as tc:
        with (
            tc.tile_pool(name="const", bufs=1) as cp,
            tc.tile_pool(name="bcast", bufs=1) as bp,
            tc.tile_pool(name="dscr", bufs=1, space="DRAM") as dp,
            tc.tile_pool(name="iou", bufs=2) as ip,
            tc.tile_pool(name="t16", bufs=2) as tp,
            tc.tile_pool(name="t32", bufs=2) as fp,
        ):
            # ---- per-partition target scalars ----
            tgt = cp.tile([ML, 2], F32)
            nc.sync.dma_start(tgt[:], tgt_d.ap())
            s2 = tgt[:, 0:1]
            e2 = tgt[:, 1:2]
            sc = cp.tile([ML, 3], F32)
            l2 = sc[:, 0:1]   # e2 - s2
            c2 = sc[:, 1:2]   # s2 + e2
            ne2 = sc[:, 2:3]  # -e2
            nc.vector.tensor_tensor(l2, e2, s2, OP.subtract)
            nc.vector.tensor_tensor(c2, e2, s2, OP.add)
            nc.vector.tensor_scalar(ne2, e2, -1.0, None, OP.mult)

            # ---- compact om load + bf16 components, bounced via DRAM ----
            omc = cp.tile([128, 2 * PK], F32)
            nc.sync.dma_start(omc[:], omc_d.ap())
            omc3 = omc[:].rearrange("p (c t) -> p c t", t=2)
            s1c = cp.tile([128, PK], BF16)
            e1c = cp.tile([128, PK], BF16)
            nc.vector.tensor_copy(s1c[:], omc3[:, :, 0])
            nc.vector.tensor_copy(e1c[:], omc3[:, :, 1])
            s1scr = dp.tile([128, PK], BF16)
            e1scr = dp.tile([128, PK], BF16)
            dma_eng = nc.sync if os.environ.get("BBK_SYNCDMA") else nc.gpsimd
            dma_eng.dma_start(s1scr[:], s1c[:])
            dma_eng.dma_start(e1scr[:], e1c[:])

            # ---- broadcast 16x across partitions ----
            # partition layout: partition p = t*8 + s handles target
            # m = s*16 + t (host permutes iou/tgt shards to match), so each
            # broadcast DMA writes a contiguous 8-partition block.
            S1B = bp.tile([128, P], BF16)
            E1B = bp.tile([128, P], BF16)
            for dst, scr in ((S1B, s1scr), (E1B, e1scr)):
                flat = scr[:].rearrange("(s k) c -> s (k c)", k=T)  # [8, 8256] DRAM
                for t in range(T):
                    dma_eng.dma_start(dst[8 * t : 8 * (t + 1), :], flat)

            # ---- accumulators ----
            acc = cp.tile([ML, 3 * NCH], F32)

            # ---- main chunked pipeline ----
            stage = int(os.environ.get("BBK_STAGE", "9"))
            for c in range(NCH):
                p0 = c * CH
                sl = slice(p0, p0 + CH)
                iou = ip.tile([ML, CH], F32)
                # triu rows intersecting this chunk, in p-order
                for i, off, ln in rows:
                    lo = max(off, p0)
                    hi = min(off + ln, p0 + CH)
                    if lo >= hi:
                        continue
                    j0 = i + (lo - off)
                    src = iou_d.ap()[:, i * N + j0 : i * N + j0 + (hi - lo)]
                    nc.sync.dma_start(iou[:, lo - p0 : hi - p0], src)

                s1 = S1B[:, sl]
                e1 = E1B[:, sl]

                mb = tp.tile([ML, CH], BF16, tag="mb")
                nc.vector.tensor_scalar(
                    mb[:], iou[:], IOU_THRESHOLD, None, OP.is_gt, OP.add,
                    accum_out=acc[:, c : c + 1],
                )
                if stage < 2:
                    continue
                u = tp.tile([ML, CH], BF16, tag="u")
                nc.vector.tensor_scalar(u[:], s1, s2, None, OP.max)
                v = tp.tile([ML, CH], BF16, tag="v")
                nc.vector.tensor_scalar(v[:], e1, e2, None, OP.min)
                w = tp.tile([ML, CH], BF16, tag="w")
                nc.vector.scalar_tensor_tensor(
                    w[:], v[:], 0.0, u[:], OP.add, OP.subtract
                )
                inter = tp.tile([ML, CH], BF16, tag="inter")
                nc.vector.tensor_scalar(inter[:], w[:], 0.0, None, OP.max)

                if stage < 3:
                    continue
                r1 = tp.tile([ML, CH], BF16, tag="r1")
                nc.scalar.activation(r1[:], e1, AF.Relu, bias=ne2, scale=1.0)
                r2 = tp.tile([ML, CH], BF16, tag="r2")
                nc.scalar.activation(r2[:], s1, AF.Relu, bias=s2, scale=-1.0)
                encb = tp.tile([ML, CH], BF16, tag="encb")
                nc.vector.scalar_tensor_tensor(
                    encb[:], r1[:], l2, r2[:], OP.add, OP.add
                )
                if stage < 4:
                    continue
                # renc = 1/enc via bitwise-not seed + one bf16 Newton pass
                nx = tp.tile([ML, CH], BF16, tag="nx")
                nc.vector.tensor_scalar(
                    nx[:].bitcast(U16), encb[:].bitcast(U16), 0, None,
                    OP.bitwise_not,
                )
                y0 = tp.tile([ML, CH], BF16, tag="y0")
                nc.vector.tensor_scalar(y0[:], nx[:], RECIP_C0, None, OP.mult)
                ay = tp.tile([ML, CH], BF16, tag="ay")
                nc.vector.tensor_tensor(ay[:], encb[:], y0[:], OP.mult)
                bq = tp.tile([ML, CH], BF16, tag="bq")
                nc.vector.tensor_scalar(bq[:], ay[:], -1.0, RECIP_C1, OP.mult, OP.add)
                renc = tp.tile([ML, CH], BF16, tag="renc")
                nc.vector.tensor_tensor(renc[:], y0[:], bq[:], OP.mult)
                if stage < 5:
                    continue
                rm = tp.tile([ML, CH], BF16, tag="rm")
                nc.vector.tensor_tensor(rm[:], mb[:], renc[:], OP.mult)
                t1 = tp.tile([ML, CH], BF16, tag="t1")
                nc.vector.scalar_tensor_tensor(
                    t1[:], inter[:], 0.0, rm[:], OP.add, OP.mult,
                    accum_out=acc[:, NCH + c : NCH + c + 1],
                )
                if stage < 6:
                    continue
                # 2*cd = s1 + e1 - c2   (B2q = sum((2cd*rm)^2) = 4*B2)
                cd2x = tp.tile([ML, CH], BF16, tag="cd2x")
                nc.vector.scalar_tensor_tensor(
                    cd2x[:], e1, c2, s1, OP.subtract, OP.add
                )
                q = tp.tile([ML, CH], BF16, tag="q")
                nc.vector.tensor_tensor(q[:], cd2x[:], rm[:], OP.mult)
                if stage < 7:
                    continue
                t2 = tp.tile([ML, CH], BF16, tag="t2")
                nc.vector.scalar_tensor_tensor(
                    t2[:], q[:], 0.0, q[:], OP.add, OP.mult,
                    accum_out=acc[:, 2 * NCH + c : 2 * NCH + c + 1],
                )

            nc.sync.dma_start(acc_d.ap(), acc[:])

    nc.compile()
    return nc


_NC_CACHE = None


def _get_program():
    global _NC_CACHE
    if _NC_CACHE is None:
        _NC_CACHE = _build_program()
    return _NC_CACHE


def _reference_numpy(out_moments, tgt_moments, num_targets, iou2ds, mask2d):
    """Exact numpy replica of the jax reference (fallback path)."""
    M_, N_, _ = iou2ds.shape
    S_, P_, _ = out_moments.shape
    scatter = np.repeat(np.arange(S_), num_targets)
    om = out_moments[scatter].astype(np.float32)      # [M, P, 2]
    tg = tgt_moments[:, None, :].astype(np.float32)
    s1, e1 = om[..., 0], om[..., 1]
    s2, e2 = tg[..., 0], tg[..., 1]
    inter = np.clip(np.minimum(e1, e2) - np.maximum(s1, s2), 0.0, None)
    union = (e1 - s1) + (e2 - s2) - inter
    iou = inter / union
    enclose = np.maximum(e1, e2) - np.minimum(s1, s2)
    cdist = (s1 + e1) * 0.5 - (s2 + e2) * 0.5
    bbox_diou = iou - (cdist * cdist) / (enclose * enclose)
    flat_idx = np.nonzero(mask2d.reshape(-1))[0]
    iou1 = iou2ds.reshape(M_, -1)[:, flat_idx]
    kth = np.argpartition(-iou1, TOPK - 1, axis=1)[:, :TOPK]
    target_mask = np.zeros((M_, P_), np.float32)
    target_mask[np.arange(M_)[:, None], kth] = 1.0
    target_mask = np.where(iou1 > IOU_THRESHOLD, 1.0, target_mask)
    loss = 1.0 - bbox_diou
    return np.float32((loss * target_mask).sum() / target_mask.sum())


def kernel(out_moments, tgt_moments, num_targets, iou2ds, mask2d):
    out_moments = np.asarray(out_moments, np.float32)
    tgt_moments = np.asarray(tgt_moments, np.float32)
    num_targets = np.asarray(num_targets, np.int32)
    iou2ds = np.asarray(iou2ds, np.float32)
    mask2d_np = np.asarray(mask2d)

    uniform = bool(np.all(num_targets == T))
    triu_ok = bool(
        np.array_equal(mask2d_np, np.triu(np.ones((N, N), dtype=bool)))
    )
    if not (uniform and triu_ok and iou2ds.shape == (M, N, N)):
        return _reference_numpy(
            out_moments, tgt_moments, num_targets, iou2ds, mask2d_np
        )

    nc = _get_program()
    # partition p on the device handles local target m = (p % W)*T + p//W
    perm = (np.arange(ML) % W) * T + np.arange(ML) // W
    in_maps = []
    for k in range(NCORES):
        iou_k = iou2ds[k * ML : (k + 1) * ML][perm]
        tgt_k = tgt_moments[k * ML : (k + 1) * ML][perm]
        in_maps.append(
            {
                "iou": np.ascontiguousarray(iou_k).reshape(ML, N * N),
                "omc": np.ascontiguousarray(
                    out_moments[k * W : (k + 1) * W]
                ).reshape(128, 2 * PK),
                "tgt": np.ascontiguousarray(tgt_k),
            }
        )

    trace = bool(int(os.environ.get("BBK_TRACE", "0")))
    res = bass_utils.run_bass_kernel_spmd(
        nc, in_maps, core_ids=list(range(NCORES)), trace=trace
    )
    if trace:
        kernel.last_exec_time_ns = res.exec_time_ns

    acc = np.stack([res.results[k]["acc"] for k in range(NCORES)])  # [8,128,24]
    acc64 = acc.astype(np.float64)
    a_rows = acc64[:, :, 0:NCH].sum(axis=2)        # per-core per-row counts
    A = a_rows.sum()
    B1 = acc64[:, :, NCH : 2 * NCH].sum() * RECIP_BIAS
    B2 = acc64[:, :, 2 * NCH : 3 * NCH].sum() / 4.0 * RECIP_BIAS**2

    if a_rows.min() < TOPK:
        # top-3 not subsumed by the threshold for some row: replicate the
        # reference exactly on host (rare/degenerate inputs only).
        return _reference_numpy(
            out_moments, tgt_moments, num_targets, iou2ds, mask2d_np
        )

    return np.float32((A - B1 + B2) / A)


# revision 16
# speedup vs baseline: 1.1999x; 1.1999x over previous
"""Bass/Trainium2 kernel for nn_BboxIoULoss (topk_masking).

Computes, for S=64 samples / M=1024 targets / P=8256 triu proposals:
    loss = sum((1 - diou) * mask) / sum(mask)
where mask = topk-scatter(3) OR (iou1ds > 0.5), iou1ds = triu-gather of
iou2ds, and diou is the 1-D DIoU between each target and the per-sample
proposal moments.

Strategy (8 NeuronCores, data-parallel over M):
  - core k handles targets m in [128k, 128(k+1)) -> 128 partitions = targets.
  - iou2ds shard is loaded triu-row-wise directly into "p-order" columns,
    so column p of the on-chip tile pairs with out_moments[s, p].
  - out_moments shard (8 samples) is loaded compactly as [128, 1032] f32,
    converted to bf16 s1/e1 [128, 516] on-chip, bounced through a DRAM
    scratch, and read back replicated 16x across partitions so that
    partition m holds sample floor(m/16).
  - per-chunk DVE/ACT pipeline (bf16) with per-partition tgt scalars:
        w     = min(e1,e2) - max(s1,s2)
        enc   = relu(e1-e2) + relu(s2-s1) + (e2-s2)    # enclose length
        inter = relu(w)
        renc  = 1/enc  (RECIPROCAL_APPROX_FAST, ~51 ULP)
        mask  = iou1 > 0.5   (exact fp32 compare)
        rm    = mask * renc
        A  += sum(mask); B1 += sum(inter*rm); B2q += sum(((s1+e1-c2)*rm)^2)
    Using mask^2 = mask:  sum(mask*diou) = B1 - B2q/4, so
        answer = (A - B1 + B2q/4) / A.
  - the top-3 scatter is subsumed by the threshold whenever every row has
    >= 3 entries above 0.5 (then the top-3 values are all > 0.5). The
    device returns per-row counts; if any row has < 3, or num_targets is
    not uniform, a numpy fallback reproduces the reference exactly.
"""

import os
import numpy as np

import concourse.bass as bass
import concourse.tile as tile
import concourse.mybir as mybir
from concourse import bacc, bass_utils

F32 = mybir.dt.float32
BF16 = mybir.dt.bfloat16
U16 = mybir.dt.uint16
AF = mybir.ActivationFunctionType
OP = mybir.AluOpType

S = 64
T = 16
N = 128
M = S * T                  # 1024
P = N * (N + 1) // 2       # 8256
TOPK = 3
IOU_THRESHOLD = 0.5
NCORES = 8
ML = M // NCORES           # 128 targets / core  (= partitions)
W = S // NCORES            # 8 samples / core
CH = 1032                  # p-chunk (P = 8 * 1032)
NCH = P // CH
PK = P // T                # 516
RECIP_C0 = -0.23549792     # Chebyshev seed scale for ~x reciprocal trick
RECIP_C1 = 2.0017324
RECIP_BIAS = 1.00138       # host-side correction for 1-NR underestimate


def _triu_rows():
    """(i, p_offset, row_len) for each triangle row in p-order."""
    rows = []
    off = 0
    for i in range(N):
        rows.append((i, off, N - i))
        off += N - i
    return rows


def _build_program():
    nc = bacc.Bacc(
        "TRN2", target_bir_lowering=False, debug=False, enable_asserts=False
    )
    # Host passes pre-reshaped shards:
    #   iou: [ML, N*N] f32   (iou2ds[m] flattened row-major)
    #   omc: [128, 1032] f32 (out_moments[8, 8256, 2] flat; row s*16+k holds
    #                         interleaved (s,e) pairs for p in [516k, 516(k+1)))
    #   tgt: [ML, 2]    f32
    iou_d = nc.dram_tensor("iou", [ML, N * N], F32, kind="ExternalInput")
    omc_d = nc.dram_tensor("omc", [128, 2 * PK], F32, kind="ExternalInput")
    tgt_d = nc.dram_tensor("tgt", [ML, 2], F32, kind="ExternalInput")
    # acc: cols [0:NCH]=A, [NCH:2NCH]=B1, [2NCH:3NCH]=B2q  (per-chunk sums)
    acc_d = nc.dram_tensor("acc", [ML, 3 * NCH], F32, kind="ExternalOutput")

    rows = _triu_rows()

    linearize = bool(int(os.environ.get("BBK_LINEARIZE", "0")))
    with tile.TileContext(nc, linearize=linearize) as tc:
        with (
            tc.tile_pool(name="const", bufs=1) as cp,
            tc.tile_pool(name="bcast", bufs=1) as bp,
            tc.tile_pool(name="dscr", bufs=1, space="DRAM") as dp,
            tc.tile_pool(name="iou", bufs=2) as ip,
            tc.tile_pool(name="ioup", bufs=1) as ipp,
            tc.tile_pool(name="t16", bufs=2) as tp,
            tc.tile_pool(name="t32", bufs=2) as fp,
        ):
            # ---- per-partition target scalars ----
            tgt = cp.tile([ML, 2], F32)
            nc.sync.dma_start(tgt[:], tgt_d.ap())
            s2 = tgt[:, 0:1]
            e2 = tgt[:, 1:2]
            sc = cp.tile([ML, 3], F32)
            l2 = sc[:, 0:1]   # e2 - s2
            c2 = sc[:, 1:2]   # s2 + e2
            ne2 = sc[:, 2:3]  # -e2
            nc.vector.tensor_tensor(l2, e2, s2, OP.subtract)
            nc.vector.tensor_tensor(c2, e2, s2, OP.add)
            nc.vector.tensor_scalar(ne2, e2, -1.0, None, OP.mult)

            # ---- compact om load + bf16 components, bounced via DRAM ----
            omc = cp.tile([128, 2 * PK], F32)
            nc.sync.dma_start(omc[:], omc_d.ap())
            omc3 = omc[:].rearrange("p (c t) -> p c t", t=2)
            s1c = cp.tile([128, PK], BF16)
            e1c = cp.tile([128, PK], BF16)
            nc.vector.tensor_copy(s1c[:], omc3[:, :, 0])
            nc.vector.tensor_copy(e1c[:], omc3[:, :, 1])
            s1scr = dp.tile([128, PK], BF16)
            e1scr = dp.tile([128, PK], BF16)
            dma_eng = nc.sync if os.environ.get("BBK_SYNCDMA") else nc.gpsimd
            dma_eng.dma_start(s1scr[:], s1c[:])
            dma_eng.dma_start(e1scr[:], e1c[:])

            # ---- broadcast 16x across partitions ----
            # partition layout: partition p = t*8 + s handles target
            # m = s*16 + t (host permutes iou/tgt shards to match), so each
            # broadcast DMA writes a contiguous 8-partition block.
            S1B = bp.tile([128, P], BF16)
            E1B = bp.tile([128, P], BF16)
            for dst, scr in ((S1B, s1scr), (E1B, e1scr)):
                flat = scr[:].rearrange("(s k) c -> s (k c)", k=T)  # [8, 8256] DRAM
                for t in range(T):
                    dma_eng.dma_start(dst[8 * t : 8 * (t + 1), :], flat)

            # ---- accumulators ----
            acc = cp.tile([ML, 3 * NCH], F32)

            # ---- iou: coalesced matrix-order staging + p-order compaction ----
            # staging chunk sc holds matrix rows i in [16sc, 16sc+16) for all
            # 128 targets: [128, 2048] f32, one DMA (8KB/partition runs).
            # DVE copies (f32 -> bf16) compact each triangle row into the
            # p-order chunk tiles ioup[c] (chunk c = p in [1032c, 1032(c+1))).
            ioup = [
                ipp.tile([ML, CH], BF16, name=f"ioup{c}", tag=f"ioup{c}")
                for c in range(NCH)
            ]
            for sc in range(8):
                stg = ip.tile([ML, 2048], F32, tag="stg")
                nc.sync.dma_start(
                    stg[:], iou_d.ap()[:, sc * 2048 : (sc + 1) * 2048]
                )
                for i in range(16 * sc, 16 * sc + 16):
                    off = i * N - (i * (i - 1)) // 2  # p-offset of row i
                    ln = N - i
                    scol = (i - 16 * sc) * N + i
                    # split at compute-chunk boundaries
                    lo = off
                    while lo < off + ln:
                        c = lo // CH
                        hi = min(off + ln, (c + 1) * CH)
                        nc.vector.tensor_copy(
                            ioup[c][:, lo - c * CH : hi - c * CH],
                            stg[:, scol + (lo - off) : scol + (hi - off)],
                        )
                        lo = hi

            # ---- main chunked pipeline ----
            stage = int(os.environ.get("BBK_STAGE", "9"))
            for c in range(NCH):
                p0 = c * CH
                sl = slice(p0, p0 + CH)
                iou = ioup[c]

                s1 = S1B[:, sl]
                e1 = E1B[:, sl]

                mb = tp.tile([ML, CH], BF16, tag="mb")
                nc.vector.tensor_scalar(
                    mb[:], iou[:], IOU_THRESHOLD, None, OP.is_gt, OP.add,
                    accum_out=acc[:, c : c + 1],
                )
                if stage < 2:
                    continue
                u = tp.tile([ML, CH], BF16, tag="u")
                nc.vector.tensor_scalar(u[:], s1, s2, None, OP.max)
                v = tp.tile([ML, CH], BF16, tag="v")
                nc.vector.tensor_scalar(v[:], e1, e2, None, OP.min)
                w = tp.tile([ML, CH], BF16, tag="w")
                nc.vector.scalar_tensor_tensor(
                    w[:], v[:], 0.0, u[:], OP.add, OP.subtract
                )
                inter = tp.tile([ML, CH], BF16, tag="inter")
                nc.vector.tensor_scalar(inter[:], w[:], 0.0, None, OP.max)

                if stage < 3:
                    continue
                r1 = tp.tile([ML, CH], BF16, tag="r1")
                nc.scalar.activation(r1[:], e1, AF.Relu, bias=ne2, scale=1.0)
                r2 = tp.tile([ML, CH], BF16, tag="r2")
                nc.scalar.activation(r2[:], s1, AF.Relu, bias=s2, scale=-1.0)
                encb = tp.tile([ML, CH], BF16, tag="encb")
                nc.vector.scalar_tensor_tensor(
                    encb[:], r1[:], l2, r2[:], OP.add, OP.add
                )
                if stage < 4:
                    continue
                # rm = mask/enc: bitwise-not seed (pre-masked) + one Newton pass
                nx = tp.tile([ML, CH], BF16, tag="nx")
                nc.vector.tensor_scalar(
                    nx[:].bitcast(U16), encb[:].bitcast(U16), 0, None,
                    OP.bitwise_not,
                )
                y0m = tp.tile([ML, CH], BF16, tag="y0m")
                nc.vector.scalar_tensor_tensor(
                    y0m[:], nx[:], RECIP_C0, mb[:], OP.mult, OP.mult
                )
                ay = tp.tile([ML, CH], BF16, tag="ay")
                nc.vector.tensor_tensor(ay[:], encb[:], y0m[:], OP.mult)
                bq = tp.tile([ML, CH], BF16, tag="bq")
                nc.vector.tensor_scalar(bq[:], ay[:], -1.0, RECIP_C1, OP.mult, OP.add)
                rm = tp.tile([ML, CH], BF16, tag="rm")
                nc.vector.tensor_tensor(rm[:], y0m[:], bq[:], OP.mult)
                if stage < 5:
                    continue
                t1 = tp.tile([ML, CH], BF16, tag="t1")
                nc.vector.scalar_tensor_tensor(
                    t1[:], inter[:], 0.0, rm[:], OP.add, OP.mult,
                    accum_out=acc[:, NCH + c : NCH + c + 1],
                )
                if stage < 6:
                    continue
                # 2*cd = s1 + e1 - c2   (B2q = sum((2cd*rm)^2) = 4*B2)
                cd2x = tp.tile([ML, CH], BF16, tag="cd2x")
                nc.vector.scalar_tensor_tensor(
                    cd2x[:], e1, c2, s1, OP.subtract, OP.add
                )
                q = tp.tile([ML, CH], BF16, tag="q")
                nc.vector.tensor_tensor(q[:], cd2x[:], rm[:], OP.mult)
                if stage < 7:
                    continue
                t2 = tp.tile([ML, CH], BF16, tag="t2")
                nc.scalar.activation(
                    t2[:], q[:], AF.Square,
                    accum_out=acc[:, 2 * NCH + c : 2 * NCH + c + 1],
                )

            nc.sync.dma_start(acc_d.ap(), acc[:])

    nc.compile()
    return nc


_NC_CACHE = None


def _get_program():
    global _NC_CACHE
    if _NC_CACHE is None:
        _NC_CACHE = _build_program()
    return _NC_CACHE


def _reference_numpy(out_moments, tgt_moments, num_targets, iou2ds, mask2d):
    """Exact numpy replica of the jax reference (fallback path)."""
    M_, N_, _ = iou2ds.shape
    S_, P_, _ = out_moments.shape
    scatter = np.repeat(np.arange(S_), num_targets)
    om = out_moments[scatter].astype(np.float32)      # [M, P, 2]
    tg = tgt_moments[:, None, :].astype(np.float32)
    s1, e1 = om[..., 0], om[..., 1]
    s2, e2 = tg[..., 0], tg[..., 1]
    inter = np.clip(np.minimum(e1, e2) - np.maximum(s1, s2), 0.0, None)
    union = (e1 - s1) + (e2 - s2) - inter
    iou = inter / union
    enclose = np.maximum(e1, e2) - np.minimum(s1, s2)
    cdist = (s1 + e1) * 0.5 - (s2 + e2) * 0.5
    bbox_diou = iou - (cdist * cdist) / (enclose * enclose)
    flat_idx = np.nonzero(mask2d.reshape(-1))[0]
    iou1 = iou2ds.reshape(M_, -1)[:, flat_idx]
    kth = np.argpartition(-iou1, TOPK - 1, axis=1)[:, :TOPK]
    target_mask = np.zeros((M_, P_), np.float32)
    target_mask[np.arange(M_)[:, None], kth] = 1.0
    target_mask = np.where(iou1 > IOU_THRESHOLD, 1.0, target_mask)
    loss = 1.0 - bbox_diou
    return np.float32((loss * target_mask).sum() / target_mask.sum())


def kernel(out_moments, tgt_moments, num_targets, iou2ds, mask2d):
    out_moments = np.asarray(out_moments, np.float32)
    tgt_moments = np.asarray(tgt_moments, np.float32)
    num_targets = np.asarray(num_targets, np.int32)
    iou2ds = np.asarray(iou2ds, np.float32)
    mask2d_np = np.asarray(mask2d)

    uniform = bool(np.all(num_targets == T))
    triu_ok = bool(
        np.array_equal(mask2d_np, np.triu(np.ones((N, N), dtype=bool)))
    )
    if not (uniform and triu_ok and iou2ds.shape == (M, N, N)):
        return _reference_numpy(
            out_moments, tgt_moments, num_targets, iou2ds, mask2d_np
        )

    nc = _get_program()
    # partition p on the device handles local target m = (p % W)*T + p//W
    perm = (np.arange(ML) % W) * T + np.arange(ML) // W
    in_maps = []
    for k in range(NCORES):
        iou_k = iou2ds[k * ML : (k + 1) * ML][perm]
        tgt_k = tgt_moments[k * ML : (k + 1) * ML][perm]
        in_maps.append(
            {
                "iou": np.ascontiguousarray(iou_k).reshape(ML, N * N),
                "omc": np.ascontiguousarray(
                    out_moments[k * W : (k + 1) * W]
                ).reshape(128, 2 * PK),
                "tgt": np.ascontiguousarray(tgt_k),
            }
        )

    trace = bool(int(os.environ.get("BBK_TRACE", "0")))
    res = bass_utils.run_bass_kernel_spmd(
        nc, in_maps, core_ids=list(range(NCORES)), trace=trace
    )
    if trace:
        kernel.last_exec_time_ns = res.exec_time_ns

    acc = np.stack([res.results[k]["acc"] for k in range(NCORES)])  # [8,128,24]
    acc64 = acc.astype(np.float64)
    a_rows = acc64[:, :, 0:NCH].sum(axis=2)        # per-core per-row counts
    A = a_rows.sum()
    B1 = acc64[:, :, NCH : 2 * NCH].sum() * RECIP_BIAS
    B2 = acc64[:, :, 2 * NCH : 3 * NCH].sum() / 4.0 * RECIP_BIAS**2

    if a_rows.min() < TOPK:
        # top-3 not subsumed by the threshold for some row: replicate the
        # reference exactly on host (rare/degenerate inputs only).
        return _reference_numpy(
            out_moments, tgt_moments, num_targets, iou2ds, mask2d_np
        )

    return np.float32((A - B1 + B2) / A)
